# revision 9
# baseline (speedup 1.0000x reference)
"""Trainium2 Bass kernel for nn_AttentionSingleCritic.

Reference computation (see problem statement):
  - BatchNorm1d (training mode, biased var, affine=False) over batch dim per agent
  - shared encoder enc = leaky(xn @ W_enc + b_enc), [A=5, B, HID=256]
  - per-head selector from agent 0, keys from agents 1..4
  - logits[k,b,a] = sel_k[b] . key_{k,a}[b]
  - reg = 1e-3 * sum_k mean_{b,a} logits^2
  - critic MLP on enc[0]: all_q = leaky(enc0 @ W_c1 + b_c1) @ W_c2 + b_c2
  Returns (all_q [B,8], reg scalar).

Dead code in the reference (multiplied by 0.0): softmax weights, vals
(W_val/b_val path), other_vals. They are skipped entirely here — exact
zero contribution to outputs.

Strategy: pure data parallelism over B across 8 cores (4096 each). BN
statistics are computed per-shard with bn_stats/bn_aggr and merged with an
on-device AllReduce of (mean, mean-square) per (agent, state-dim). The BN
normalization is folded into the encoder weights (W' = inv_sigma * W_enc,
b' = b_enc - (mu*inv_sigma) @ W_enc), so states are read exactly once.

Layout: everything runs "transposed" — activations are [feature, batch]
(feature on SBUF partitions, batch on the free dim), so every matmul
contracts over partitions with the weight stationary and batch moving with
free-dim 512. States are pre-transposed on the host into [3, 128, B] with
agent pairs stacked at partition offsets 0 and 64.

Matmuls run in float32r (full PE rate for free-dim >= 256).
"""

import sys

sys.path.insert(0, "/opt/trn_rl_repo")

import numpy as np

A, B, S, HID, HEADS, D, ADIM = 5, 32768, 48, 256, 4, 64, 8
NCORES = 8
BS = B // NCORES          # 4096 batch per core
BT = 512                  # batch tile (matmul moving free dim)
NBT = BS // BT            # 8 batch tiles per core
NG = 3                    # agent pair groups (0,1) (2,3) (4,-)
LRELU = 0.01
BN_EPS = 1e-5

_CACHED = {}


def _build_nc(use_act_lrelu=True):
    import concourse.bacc as bacc
    import concourse.tile as tile
    import concourse.mybir as mybir

    F32 = mybir.dt.float32
    F32R = mybir.dt.float32r
    BF16 = mybir.dt.bfloat16
    AF = mybir.ActivationFunctionType
    OP = mybir.AluOpType
    AX = mybir.AxisListType

    nc = bacc.Bacc("TRN2", target_bir_lowering=False, debug=False,
                   num_devices=NCORES)

    # ---- I/O -----------------------------------------------------------
    xT_d = nc.dram_tensor("xt", [NG, 128, BS], F32R, kind="ExternalInput")
    wenc_d = nc.dram_tensor("w_enc", [S, HID], F32, kind="ExternalInput")
    benc_d = nc.dram_tensor("b_enc_t", [128, 2], F32, kind="ExternalInput")
    wsel_d = nc.dram_tensor("w_sel", [128, 2, HID], F32R, kind="ExternalInput")
    wkey_d = nc.dram_tensor("w_key", [128, 2, HID], F32R, kind="ExternalInput")
    wc1_d = nc.dram_tensor("w_c1", [128, 2, HID], F32R, kind="ExternalInput")
    bc1_d = nc.dram_tensor("b_c1_t", [128, 2], F32, kind="ExternalInput")
    wc2_d = nc.dram_tensor("w_c2", [128, 2, ADIM], F32R, kind="ExternalInput")
    bc2_d = nc.dram_tensor("b_c2", [ADIM, 1], F32, kind="ExternalInput")
    seg_d = nc.dram_tensor("seg", [128, 32], BF16, kind="ExternalInput")
    outq_d = nc.dram_tensor("all_q_t", [ADIM, BS], F32, kind="ExternalOutput")
    outr_d = nc.dram_tensor("reg_part", [128, 1], F32, kind="ExternalOutput")

    with tile.TileContext(nc) as tc:
        with tc.tile_pool(name="per", bufs=1) as per, \
             tc.tile_pool(name="enc", bufs=3) as ep, \
             tc.tile_pool(name="small", bufs=2) as sp, \
             tc.tile_pool(name="ps", bufs=2, space="PSUM") as pp, \
             tc.tile_pool(name="dram", bufs=1, space="DRAM") as dp:

            # ---- load inputs ------------------------------------------
            xt = []
            for i in range(NG):
                t = per.tile([128, BS], F32R, tag=f"xt{i}")
                nc.sync.dma_start(out=t[:], in_=xT_d[i])
                xt.append(t)
            wenc = per.tile([128, HID], F32)
            nc.vector.memset(wenc[:], 0.0)
            nc.sync.dma_start(out=wenc[0:S, :], in_=wenc_d[:])
            nc.sync.dma_start(out=wenc[64:64 + S, :], in_=wenc_d[:])
            wsel = per.tile([128, 2, HID], F32R)
            nc.sync.dma_start(out=wsel[:], in_=wsel_d[:])
            wkey = per.tile([128, 2, HID], F32R)
            nc.sync.dma_start(out=wkey[:], in_=wkey_d[:])
            wc1 = per.tile([128, 2, HID], F32R)
            nc.sync.dma_start(out=wc1[:], in_=wc1_d[:])
            wc2 = per.tile([128, 2, ADIM], F32R)
            nc.sync.dma_start(out=wc2[:], in_=wc2_d[:])
            benc = per.tile([128, 2], F32)
            nc.sync.dma_start(out=benc[:], in_=benc_d[:])
            bc1 = per.tile([128, 2], F32)
            nc.sync.dma_start(out=bc1[:], in_=bc1_d[:])
            bc2 = per.tile([ADIM, 1], F32)
            nc.sync.dma_start(out=bc2[:], in_=bc2_d[:])
            seg = per.tile([128, 32], BF16)
            nc.sync.dma_start(out=seg[:], in_=seg_d[:])

            # ---- per-shard BN stats -----------------------------------
            # bn_stats is limited to 512 free elems per call.
            arin = per.tile([128, 2 * NG], F32)   # (mean, msq) per group
            for i in range(NG):
                bnst = sp.tile([128, NBT, 6], F32, tag="bnst")
                for g in range(NBT):
                    nc.vector.bn_stats(out=bnst[:, g, :],
                                       in_=xt[i][:, g * BT:(g + 1) * BT].bitcast(F32))
                mv = sp.tile([128, 2], F32, tag="mv")
                nc.vector.bn_aggr(out=mv[:], in_=bnst[:])
                nc.vector.tensor_copy(arin[:, 2 * i:2 * i + 1], mv[:, 0:1])
                # msq = mean^2 + var
                nc.vector.scalar_tensor_tensor(
                    out=arin[:, 2 * i + 1:2 * i + 2], in0=mv[:, 0:1],
                    scalar=mv[:, 0:1], in1=mv[:, 1:2], op0=OP.mult, op1=OP.add)

            # ---- AllReduce stats across the 8 cores -------------------
            cc_in = dp.tile([128, 2 * NG], F32)
            cc_out = dp.tile([128, 2 * NG], F32)
            nc.sync.dma_start(out=cc_in[:], in_=arin[:])
            nc.gpsimd.collective_compute(
                "AllReduce", OP.add,
                replica_groups=[list(range(NCORES))],
                ins=[cc_in.opt()], outs=[cc_out.opt()])
            gsum = per.tile([128, 2 * NG], F32)
            nc.sync.dma_start(out=gsum[:], in_=cc_out[:])

            # ---- fold BN into encoder weights -------------------------
            # mean_g = sum/8 ; var_g = msq_sum/8 - mean_g^2
            c_inv64 = per.tile([128, 1], F32)
            nc.vector.memset(c_inv64[:], 1.0 / 64.0)
            eps = per.tile([128, 1], F32)
            nc.vector.memset(eps[:], BN_EPS)
            mean8 = per.tile([128, NG], F32)
            wps = []
            for i in range(NG):
                nc.vector.tensor_scalar_mul(mean8[:, i:i + 1],
                                            gsum[:, 2 * i:2 * i + 1], 0.125)
                sq = sp.tile([128, 1], F32, tag="sq")
                nc.vector.scalar_tensor_tensor(
                    out=sq[:], in0=gsum[:, 2 * i:2 * i + 1],
                    scalar=gsum[:, 2 * i:2 * i + 1], in1=c_inv64[:, 0:1],
                    op0=OP.mult, op1=OP.mult)
                var = sp.tile([128, 1], F32, tag="var")
                nc.vector.scalar_tensor_tensor(
                    out=var[:], in0=gsum[:, 2 * i + 1:2 * i + 2], scalar=0.125,
                    in1=sq[:, 0:1], op0=OP.mult, op1=OP.subtract)
                sig = sp.tile([128, 1], F32, tag="sig")
                nc.scalar.activation(out=sig[:], in_=var[:], func=AF.Sqrt,
                                     bias=eps[:, 0:1])
                isig = sp.tile([128, 1], F32, tag="isig")
                nc.vector.reciprocal(isig[:], sig[:])
                wp = per.tile([128, HID], F32R, tag=f"wp{i}")
                nc.vector.tensor_scalar_mul(wp[:], wenc[:], isig[:, 0:1])
                wps.append(wp)

            # folded bias, transposed: b'_am = b_enc_m - Wp[:, m].T @ mean
            # (one tiny matmul per (agent, chunk))
            bpr = per.tile([128, A, 2], F32)
            mean8r = per.tile([128, NG], F32R)
            nc.vector.tensor_copy(mean8r[:], mean8[:])
            for a in range(A):
                i, off = divmod(a, 2)
                sr = slice(64 * off, 64 * off + S)
                for m in range(2):
                    pt = pp.tile([128, 1], F32, tag="qps")
                    nc.tensor.matmul(pt[:],
                                     wps[i][sr, m * 128:(m + 1) * 128].bitcast(F32),
                                     mean8r[sr, i:i + 1].bitcast(F32),
                                     start=True, stop=True)
                    nc.vector.scalar_tensor_tensor(
                        out=bpr[:, a, m:m + 1], in0=pt[:], scalar=-1.0,
                        in1=benc[:, m:m + 1], op0=OP.mult, op1=OP.add)

            # ---- main fused loop over batch tiles ---------------------
            enc0 = per.tile([128, 2, BS], F32R)        # agent-0 encoding
            regacc = per.tile([128, NBT * 2], F32)     # ACT square accums
            nc.vector.memset(regacc[:], 0.0)

            def enc_mm(a, m, bsl, psb):
                i, off = divmod(a, 2)
                sr = slice(64 * off, 64 * off + S)
                nc.tensor.matmul(psb[:], wps[i][sr, m * 128:(m + 1) * 128],
                                 xt[i][sr, bsl], start=True, stop=True)

            def leaky_act(dst, src_ps, bias_ap):
                # dst = leaky(src + bias); ACT Lrelu or DVE 2-op fallback
                if use_act_lrelu:
                    nc.scalar.activation(out=dst, in_=src_ps, func=AF.Lrelu,
                                         bias=bias_ap, alpha=LRELU)
                else:
                    tmp = ep.tile([128, BT], F32, tag="lk")
                    nc.vector.tensor_scalar_add(tmp[:], src_ps, bias_ap)
                    nc.vector.scalar_tensor_tensor(
                        out=dst, in0=tmp[:], scalar=LRELU, in1=tmp[:],
                        op0=OP.mult, op1=OP.max)

            for bt in range(NBT):
                bsl = slice(bt * BT, (bt + 1) * BT)
                # agent-0 encoding (persistent, reused by sel/critic)
                for m in range(2):
                    psb = pp.tile([128, BT], F32, tag="big")
                    enc_mm(0, m, bsl, psb)
                    leaky_act(enc0[:, m, bsl], psb[:], bpr[:, 0, m:m + 1])
                # selector, transposed [d4, b]
                selT = ep.tile([128, 2, BT], F32, tag="selT")
                for c in range(2):
                    ps_sel = pp.tile([128, BT], F32, tag="big")
                    for m in range(2):
                        nc.tensor.matmul(ps_sel[:], wsel[:, m, c * 128:(c + 1) * 128],
                                         enc0[:, m, bsl], start=(m == 0), stop=(m == 1))
                    nc.vector.tensor_copy(selT[:, c, :], ps_sel[:])
                # per non-zero agent: encoding, keys, products, head-reduce, square
                ps_reg = None
                for a in range(1, A):
                    et = ep.tile([128, 2, BT], F32R, tag="encA")
                    for m in range(2):
                        psb = pp.tile([128, BT], F32, tag="big")
                        enc_mm(a, m, bsl, psb)
                        if a >= 3:
                            tmp = ep.tile([128, BT], F32, tag="lk")
                            nc.vector.tensor_scalar_add(tmp[:], psb[:],
                                                        bpr[:, a, m:m + 1])
                            nc.vector.scalar_tensor_tensor(
                                out=et[:, m, :], in0=tmp[:], scalar=LRELU,
                                in1=tmp[:], op0=OP.mult, op1=OP.max)
                        else:
                            leaky_act(et[:, m, :], psb[:], bpr[:, a, m:m + 1])
                    for c in range(2):
                        ps_k = pp.tile([128, BT], F32, tag="big")
                        for m in range(2):
                            nc.tensor.matmul(ps_k[:], wkey[:, m, c * 128:(c + 1) * 128],
                                             et[:, m, :], start=(m == 0), stop=(m == 1))
                        prod = ep.tile([128, BT], BF16, tag="prod")
                        nc.vector.scalar_tensor_tensor(
                            out=prod[:], in0=ps_k[:], scalar=1.0,
                            in1=selT[:, c, :], op0=OP.mult, op1=OP.mult)
                        # pack 4 [2,BT] head-sum pieces into one PSUM bank via
                        # col-tiling, then one Square+accum over the full bank
                        jj = (a - 1) * 2 + c
                        slot = jj % 4
                        if slot == 0:
                            ps_reg = pp.tile([128, BT], F32, tag="logps")
                        nc.tensor.matmul(ps_reg[32 * slot:32 * slot + 32, :],
                                         seg[:], prod[:], start=True, stop=True,
                                         tile_position=(0, 32 * slot))
                        if slot == 3:
                            sqs = ep.tile([128, BT], F32, tag="sqs")
                            col = bt * 2 + jj // 4
                            nc.scalar.activation(out=sqs[:], in_=ps_reg[:],
                                                 func=AF.Square,
                                                 accum_out=regacc[:, col:col + 1])
                # critic
                hT = ep.tile([128, 2, BT], F32R, tag="hT")
                for mo in range(2):
                    ps_h = pp.tile([128, BT], F32, tag="big")
                    for m in range(2):
                        nc.tensor.matmul(ps_h[:], wc1[:, m, mo * 128:(mo + 1) * 128],
                                         enc0[:, m, bsl], start=(m == 0), stop=(m == 1))
                    leaky_act(hT[:, mo, :], ps_h[:], bc1[:, mo:mo + 1])
                ps_q = pp.tile([ADIM, BT], F32, tag="qps")
                for m in range(2):
                    nc.tensor.matmul(ps_q[:], wc2[:, m, :], hT[:, m, :],
                                     start=(m == 0), stop=(m == 1))
                qsb = ep.tile([ADIM, BT], F32, tag="qsb")
                nc.vector.tensor_scalar_add(qsb[:], ps_q[:], bc2[:, 0:1])
                nc.sync.dma_start(out=outq_d[:, bsl], in_=qsb[:])

            # ---- finalize reg partial ---------------------------------
            regout = per.tile([128, 1], F32)
            nc.vector.reduce_sum(out=regout[:], in_=regacc[:], axis=AX.X)
            nc.sync.dma_start(out=outr_d[:], in_=regout[:])

    nc.finalize()
    return nc


def _host_prep(inputs):
    states = np.asarray(inputs["states"], np.float32)
    W_enc = np.asarray(inputs["W_enc"], np.float32)
    b_enc = np.asarray(inputs["b_enc"], np.float32)
    W_sel = np.asarray(inputs["W_sel"], np.float32)
    W_key = np.asarray(inputs["W_key"], np.float32)
    W_c1 = np.asarray(inputs["W_c1"], np.float32)
    b_c1 = np.asarray(inputs["b_c1"], np.float32)
    W_c2 = np.asarray(inputs["W_c2"], np.float32)
    b_c2 = np.asarray(inputs["b_c2"], np.float32)

    # states [A, B, S] -> stacked transposed [NG, 128, B]
    statesT = states.transpose(0, 2, 1)  # [A, S, B]
    stk = np.zeros((NG, 128, B), np.float32)
    for i in range(NG):
        stk[i, 0:S] = statesT[2 * i]
        if 2 * i + 1 < A:
            stk[i, 64:64 + S] = statesT[2 * i + 1]

    def chunked(w):  # [256, N] -> [128, 2, N] (p, chunk, col)
        return np.ascontiguousarray(w.reshape(2, 128, -1).transpose(1, 0, 2))

    wsel_flat = W_sel.transpose(1, 0, 2).reshape(HID, HEADS * D)
    wkey_flat = W_key.transpose(1, 0, 2).reshape(HID, HEADS * D)
    import ml_dtypes
    seg = np.zeros((128, 32), np.float32)
    seg[0:64, 0] = 1.0
    seg[64:128, 1] = 1.0
    seg = seg.astype(ml_dtypes.bfloat16)

    common = {
        "w_enc": np.ascontiguousarray(W_enc),
        "b_enc_t": np.ascontiguousarray(b_enc.reshape(2, 128).T),
        "w_sel": chunked(wsel_flat),
        "w_key": chunked(wkey_flat),
        "w_c1": chunked(W_c1),
        "b_c1_t": np.ascontiguousarray(b_c1.reshape(2, 128).T),
        "w_c2": chunked(W_c2),
        "b_c2": np.ascontiguousarray(b_c2.reshape(ADIM, 1)),
        "seg": seg,
    }
    in_maps = []
    for c in range(NCORES):
        m = dict(common)
        m["xt"] = np.ascontiguousarray(stk[:, :, c * BS:(c + 1) * BS])
        in_maps.append(m)
    return in_maps


def kernel(**inputs):
    import os
    from concourse.bass_utils import run_bass_kernel_spmd

    use_act = os.environ.get("KERNEL_ACT_LRELU", "1") == "1"
    key = ("nc", use_act)
    if key not in _CACHED:
        _CACHED[key] = _build_nc(use_act_lrelu=use_act)
    nc = _CACHED[key]

    in_maps = _host_prep(inputs)
    trace = os.environ.get("KERNEL_TRACE", "0") == "1"
    kw = {}
    if trace:
        kw = dict(trace=True, trace_cores=[0])
    res = run_bass_kernel_spmd(nc, in_maps, core_ids=list(range(NCORES)), **kw)
    if trace and res.exec_time_ns is not None:
        print(f"HW exec time: {res.exec_time_ns} ns")
        _CACHED["last_results"] = res

    all_q = np.empty((B, ADIM), np.float32)
    total_sq = 0.0
    for c in range(NCORES):
        r = res.results[c]
        all_q[c * BS:(c + 1) * BS] = r["all_q_t"].T
        total_sq += float(r["reg_part"].sum())
    reg = np.float32(1e-3 * total_sq / (B * (A - 1)))
    return all_q, reg
